# revision 18
# baseline (speedup 1.0000x reference)
"""Trainium2 Bass kernel for nn_KeyFeatureExtractor.

Math: k = x @ Wk.T, split into 16 heads of 64 dims; per head a causal
exponential-decay weighted sum over sequence positions
(coef[t,s] = exp(-beta_h (t-s)) for s<=t); L2-normalize over head dim;
multiply by a per-head clamped scale.

Instead of the dense (T,T) decay matmul, we exploit the decay structure:
out = within-chunk decay matmul (chunks of L=128) + a rank-1 cross-chunk
correction dvec (x) S_j, where S_j = P_{j-1} and P is a decayed prefix sum
of per-chunk tails computed as a 4-step Kogge-Stone weighted scan on the
Vector engine. Heavy matmuls run as float32r (full-rate fp32 streaming).

Sharding: data-parallel over batch B=8 -> one batch element per NeuronCore.
"""

import sys

for _p in ("/opt/trn_rl_repo",):
    if _p not in sys.path:
        sys.path.insert(0, _p)

import numpy as np

import concourse.mybir as mybir
import concourse.tile as tile
from concourse import bacc
from concourse.alu_op_type import AluOpType
from concourse.bass_utils import run_bass_kernel_spmd

B, T, C, NH, HS = 8, 2048, 1024, 16, 64
L = 128          # chunk length (= PE contraction width)
NJ = T // L      # 16 chunks
NCI = C // 128   # 8 contraction tiles for the projection
EXP_SCALING = 10.0
KEY_SCALE_MAX = float(np.log(2 ** 16 - 1))
F32 = mybir.dt.float32
F32R = mybir.dt.float32r
ACT = mybir.ActivationFunctionType

_cache = {}
LAST_RESULTS = None  # BassKernelResults of the most recent run (for profiling)


def _r(ap):
    return ap.bitcast(F32R)


def _build(invsc2):
    """Build the per-core Bass program. invsc2[h] = 1/scale_h^2 are baked
    as activation immediates; everything else arrives as DRAM inputs."""
    nc = bacc.Bacc("TRN2", target_bir_lowering=False, debug=False)

    xT = nc.dram_tensor("xT", [C, T], F32, kind="ExternalInput")
    wkT = nc.dram_tensor("wkT", [C, C], F32, kind="ExternalInput")
    ctd = nc.dram_tensor("ct", [L, NH * L], F32, kind="ExternalInput")
    wvd = nc.dram_tensor("wv", [L, NH], F32, kind="ExternalInput")
    dvd = nc.dram_tensor("dv", [NH, NH * L], F32, kind="ExternalInput")
    cmd = nc.dram_tensor("cm", [NH, 4], F32, kind="ExternalInput")
    outd = nc.dram_tensor("out", [NH, T, HS], F32, kind="ExternalOutput")

    with tile.TileContext(nc) as tc:
        with (
            tc.tile_pool(name="wk", bufs=1) as wkp,
            tc.tile_pool(name="xt", bufs=16) as xtp,
            tc.tile_pool(name="kall", bufs=1) as kallp,
            tc.tile_pool(name="aux", bufs=1) as auxp,
            tc.tile_pool(name="norm", bufs=4) as normp,
            tc.tile_pool(name="outs", bufs=3) as outsp,
            tc.tile_pool(name="ps", bufs=3, space="PSUM") as psp,
            tc.tile_pool(name="pa", bufs=2, space="PSUM") as psap,
        ):
            ct_t = auxp.tile([L, NH * L], F32R, tag="ct")
            nc.sync.dma_start(ct_t[:], _r(ctd[:]))
            wv_t = auxp.tile([L, NH], F32R, tag="wv")
            nc.sync.dma_start(wv_t[:], _r(wvd[:]))
            dv_t = auxp.tile([NH, NH * L], F32R, tag="dv")
            nc.sync.dma_start(dv_t[:], _r(dvd[:]))
            cm_t = auxp.tile([NH, 4], F32, tag="cm")
            nc.sync.dma_start(cm_t[:], cmd[:])
            # scan ping-pong rows; [0:1024) stays zero as shift-in padding
            A0 = auxp.tile([NH, 2048], F32, tag="A0")
            A1 = auxp.tile([NH, 2048], F32, tag="A1")
            nc.vector.memset(A0[:], 0.0)
            nc.vector.memset(A1[:], 0.0)

            wk_tiles = []
            for ci in range(NCI):
                wt = wkp.tile([128, C], F32R, tag=f"wk{ci}")
                nc.sync.dma_start(wt[:], _r(wkT[ci * 128:(ci + 1) * 128, :]))
                wk_tiles.append(wt)

            # k_all[s, j, h*64+d] : projection output, row-tile j on free dim
            k_all = kallp.tile([128, NJ, NH * HS], F32R, tag="kall")

            # ---- projection: k = x @ Wk.T (lhsT = xT tile, rhs = WkT) ----
            for t in range(NJ):
                ps = psp.tile([128, 1024], F32, tag="ps")
                xts = []
                for ci in range(NCI):
                    xt_ = xtp.tile([128, 128], F32R, tag="xt")
                    nc.sync.dma_start(
                        xt_[:],
                        _r(xT[ci * 128:(ci + 1) * 128, t * 128:(t + 1) * 128]),
                    )
                    xts.append(xt_)
                for half in range(2):
                    for ci in range(NCI):
                        nc.tensor.matmul(
                            ps[:, half * 512:(half + 1) * 512],
                            _r(xts[ci][:]),
                            _r(wk_tiles[ci][:, half * 512:(half + 1) * 512]),
                            start=(ci == 0),
                            stop=(ci == NCI - 1),
                        )
                nc.scalar.copy(k_all[:, t, :], ps[:])

            # ---- a-rows: a_j = sum_s exp(-beta(127-s)) k_chunk_j[s] ----
            for h in range(NH):
                for half in range(2):
                    pa = psap.tile([1, 512], F32, tag="pa")
                    nc.tensor.matmul(
                        pa[:],
                        _r(wv_t[:, h:h + 1]),
                        _r(k_all[:, half * 8:(half + 1) * 8, h * 64:(h + 1) * 64]),
                        start=True,
                        stop=True,
                    )
                    arow = normp.tile([1, 512], F32, tag="arow")
                    nc.scalar.copy(arow[:], pa[:])
                    nc.sync.dma_start(
                        A0[h:h + 1, 1024 + half * 512:1024 + (half + 1) * 512],
                        arow[:],
                    )

            # ---- Kogge-Stone weighted scan: P_j = sum_{i<=j} dL^{j-i} a_i ----
            src, dst = A0, A1
            for m in range(4):
                shift = 64 * (2 ** m)
                nc.vector.scalar_tensor_tensor(
                    dst[:, 1024:2048],
                    src[:, 1024 - shift:2048 - shift],
                    cm_t[:, m:m + 1],
                    src[:, 1024:2048],
                    AluOpType.mult,
                    AluOpType.add,
                )
                src, dst = dst, src
            P = src  # (NH, 2048); S_j = P_{j-1} -> read at offset 960
            Pr = auxp.tile([NH, 1024], F32R, tag="Pr")
            nc.scalar.copy(Pr[:], P[:, 960:1984])

            # ---- per head: correction + within-chunk decay + normalize ----
            for h in range(NH):
                pk = psp.tile([128, NJ, HS], F32, tag="ps")
                for half in range(2):
                    nc.tensor.matmul(
                        pk[:, half * 8:(half + 1) * 8, :],
                        _r(dv_t[:, h * L:(h + 1) * L]),
                        _r(Pr[:, half * 512:(half + 1) * 512]),
                        start=True,
                        stop=False,
                    )
                    nc.tensor.matmul(
                        pk[:, half * 8:(half + 1) * 8, :],
                        _r(ct_t[:, h * 128:(h + 1) * 128]),
                        _r(k_all[:, half * 8:(half + 1) * 8, h * 64:(h + 1) * 64]),
                        start=False,
                        stop=True,
                    )
                sq = normp.tile([128, NJ, HS], F32, tag="sq")
                nc.scalar.square(sq[:], pk[:])
                ss = normp.tile([128, NJ], F32, tag="ss")
                nc.vector.tensor_reduce(
                    ss[:], sq[:], mybir.AxisListType.X, AluOpType.add
                )
                # sqrt(ss / scale_h^2) = norm / scale_h ; then reciprocal
                rns = normp.tile([128, NJ], F32, tag="rns")
                nc.scalar.activation(rns[:], ss[:], ACT.Sqrt, scale=float(invsc2[h]))
                rcp = normp.tile([128, NJ], F32, tag="rcp")
                nc.vector.reciprocal(rcp[:], rns[:])
                oh = outsp.tile([128, NJ, HS], F32, tag="oh")
                for j in range(NJ):
                    nc.vector.tensor_scalar_mul(
                        oh[:, j, :], pk[:, j, :], rcp[:, j:j + 1]
                    )
                nc.sync.dma_start(
                    outd[h].rearrange("(j r) d -> r j d", r=128), oh[:]
                )

    nc.compile()
    return nc


def _host_tensors(x, Wk_weight, betas):
    s = np.arange(L)
    diff = s[None, :] - s[:, None]  # [s, t] -> t - s
    ct = np.zeros((L, NH * L), np.float32)
    wv = np.zeros((L, NH), np.float32)
    dv = np.zeros((NH, NH * L), np.float32)  # masked: row h' nonzero iff h'==h
    cm = np.zeros((NH, 4), np.float32)
    for h in range(NH):
        b = betas[h]
        ct[:, h * L:(h + 1) * L] = np.where(diff >= 0, np.exp(-b * diff), 0.0)
        wv[:, h] = np.exp(-b * (L - 1 - s))
        dv[h, h * L:(h + 1) * L] = np.exp(-b * (s + 1.0))
        for m in range(4):
            cm[h, m] = np.exp(-b * L * (2.0 ** m))
    wkT = np.ascontiguousarray(Wk_weight.T).astype(np.float32)
    return ct, wv, dv, cm, wkT


def kernel(x, Wk_weight, leaky_key_beta, key_scale, scale_pow, **_):
    global LAST_RESULTS
    x = np.asarray(x, dtype=np.float32)
    Wk_weight = np.asarray(Wk_weight, dtype=np.float32)
    betas = (np.abs(np.asarray(leaky_key_beta, dtype=np.float64)).reshape(NH)
             * EXP_SCALING)
    ksc = np.asarray(key_scale, dtype=np.float64).reshape(NH)
    sp = float(np.asarray(scale_pow))
    scale = np.exp(np.minimum(sp * EXP_SCALING * ksc, KEY_SCALE_MAX))
    invsc2 = 1.0 / (scale * scale)

    key = (betas.tobytes(), invsc2.tobytes())
    if key not in _cache:
        _cache[key] = _build(invsc2)
    nc = _cache[key]

    ct, wv, dv, cm, wkT = _host_tensors(x, Wk_weight, betas)
    in_maps = []
    for b in range(B):
        in_maps.append({
            "xT": np.ascontiguousarray(x[b].T),
            "wkT": wkT,
            "ct": ct,
            "wv": wv,
            "dv": dv,
            "cm": cm,
        })
    res = run_bass_kernel_spmd(nc, in_maps, core_ids=list(range(B)))
    LAST_RESULTS = res
    out = np.stack([res.results[i]["out"] for i in range(B)], axis=0)
    return out


# revision 22
# speedup vs baseline: 2.0411x; 2.0411x over previous
"""Trainium2 Bass kernel for nn_KeyFeatureExtractor.

Math: k = x @ Wk.T, split into 16 heads of 64 dims; per head a causal
exponential-decay weighted sum over sequence positions
(coef[t,s] = exp(-beta_h (t-s)) for s<=t); L2-normalize over head dim;
multiply by a per-head clamped scale.

Instead of the dense (T,T) decay matmul, we exploit the decay structure:
out = within-chunk decay matmul (chunks of L=128) + a rank-1 cross-chunk
correction dvec (x) S_j, where S_j = P_{j-1} and P is a decayed prefix sum
of per-chunk tails computed as a 4-step Kogge-Stone weighted scan on the
Vector engine. Heavy matmuls run as float32r (full-rate fp32 streaming).

Sharding: data-parallel over batch B=8 -> one batch element per NeuronCore.
"""

import sys

for _p in ("/opt/trn_rl_repo",):
    if _p not in sys.path:
        sys.path.insert(0, _p)

import numpy as np

import jax
from jax.experimental.shard_map import shard_map
from jax.sharding import Mesh, PartitionSpec

import concourse.mybir as mybir
import concourse.tile as tile
from concourse import bacc
from concourse import bass2jax as b2j
from concourse.alu_op_type import AluOpType

B, T, C, NH, HS = 8, 2048, 1024, 16, 64
L = 128          # chunk length (= PE contraction width)
NJ = T // L      # 16 chunks
NCI = C // 128   # 8 contraction tiles for the projection
EXP_SCALING = 10.0
KEY_SCALE_MAX = float(np.log(2 ** 16 - 1))
F32 = mybir.dt.float32
F32R = mybir.dt.float32r
ACT = mybir.ActivationFunctionType

_cache = {}
LAST_RESULTS = None  # BassKernelResults of the most recent run (for profiling)


def _r(ap):
    return ap.bitcast(F32R)


def _build(invsc2):
    """Build the per-core Bass program. invsc2[h] = 1/scale_h^2 are baked
    as activation immediates; everything else arrives as DRAM inputs."""
    nc = bacc.Bacc("TRN2", target_bir_lowering=False, debug=False)

    xT = nc.dram_tensor("xT", [C, T], F32, kind="ExternalInput")
    wkT = nc.dram_tensor("wkT", [C, C], F32, kind="ExternalInput")
    ctd = nc.dram_tensor("ct", [L, NH * L], F32, kind="ExternalInput")
    wvd = nc.dram_tensor("wv", [L, NH], F32, kind="ExternalInput")
    dvd = nc.dram_tensor("dv", [NH, NH * L], F32, kind="ExternalInput")
    cmd = nc.dram_tensor("cm", [NH, 4], F32, kind="ExternalInput")
    outd = nc.dram_tensor("out", [NH, T, HS], F32, kind="ExternalOutput")

    with tile.TileContext(nc) as tc:
        with (
            tc.tile_pool(name="wk", bufs=1) as wkp,
            tc.tile_pool(name="xt", bufs=16) as xtp,
            tc.tile_pool(name="kall", bufs=1) as kallp,
            tc.tile_pool(name="aux", bufs=1) as auxp,
            tc.tile_pool(name="norm", bufs=4) as normp,
            tc.tile_pool(name="outs", bufs=3) as outsp,
            tc.tile_pool(name="ps", bufs=3, space="PSUM") as psp,
            tc.tile_pool(name="pa", bufs=2, space="PSUM") as psap,
        ):
            ct_t = auxp.tile([L, NH * L], F32R, tag="ct")
            nc.sync.dma_start(ct_t[:], _r(ctd[:]))
            wv_t = auxp.tile([L, NH], F32R, tag="wv")
            nc.sync.dma_start(wv_t[:], _r(wvd[:]))
            dv_t = auxp.tile([NH, NH * L], F32R, tag="dv")
            nc.sync.dma_start(dv_t[:], _r(dvd[:]))
            cm_t = auxp.tile([NH, 4], F32, tag="cm")
            nc.sync.dma_start(cm_t[:], cmd[:])
            # scan ping-pong rows; [0:1024) stays zero as shift-in padding
            A0 = auxp.tile([NH, 2048], F32, tag="A0")
            A1 = auxp.tile([NH, 2048], F32, tag="A1")
            nc.vector.memset(A0[:], 0.0)
            nc.vector.memset(A1[:], 0.0)

            wk_tiles = []
            for ci in range(NCI):
                wt = wkp.tile([128, C], F32R, tag=f"wk{ci}")
                nc.sync.dma_start(wt[:], _r(wkT[ci * 128:(ci + 1) * 128, :]))
                wk_tiles.append(wt)

            # k_all[s, j, h*64+d] : projection output, row-tile j on free dim
            k_all = kallp.tile([128, NJ, NH * HS], F32R, tag="kall")

            # ---- projection: k = x @ Wk.T (lhsT = xT tile, rhs = WkT) ----
            for t in range(NJ):
                ps = psp.tile([128, 1024], F32, tag="ps")
                xts = []
                for ci in range(NCI):
                    xt_ = xtp.tile([128, 128], F32R, tag="xt")
                    nc.sync.dma_start(
                        xt_[:],
                        _r(xT[ci * 128:(ci + 1) * 128, t * 128:(t + 1) * 128]),
                    )
                    xts.append(xt_)
                for half in range(2):
                    for ci in range(NCI):
                        nc.tensor.matmul(
                            ps[:, half * 512:(half + 1) * 512],
                            _r(xts[ci][:]),
                            _r(wk_tiles[ci][:, half * 512:(half + 1) * 512]),
                            start=(ci == 0),
                            stop=(ci == NCI - 1),
                        )
                nc.scalar.copy(k_all[:, t, :], ps[:])

            # ---- a-rows: a_j = sum_s exp(-beta(127-s)) k_chunk_j[s] ----
            for h in range(NH):
                for half in range(2):
                    pa = psap.tile([1, 512], F32, tag="pa")
                    nc.tensor.matmul(
                        pa[:],
                        _r(wv_t[:, h:h + 1]),
                        _r(k_all[:, half * 8:(half + 1) * 8, h * 64:(h + 1) * 64]),
                        start=True,
                        stop=True,
                    )
                    arow = normp.tile([1, 512], F32, tag="arow")
                    nc.scalar.copy(arow[:], pa[:])
                    nc.sync.dma_start(
                        A0[h:h + 1, 1024 + half * 512:1024 + (half + 1) * 512],
                        arow[:],
                    )

            # ---- Kogge-Stone weighted scan: P_j = sum_{i<=j} dL^{j-i} a_i ----
            src, dst = A0, A1
            for m in range(4):
                shift = 64 * (2 ** m)
                nc.vector.scalar_tensor_tensor(
                    dst[:, 1024:2048],
                    src[:, 1024 - shift:2048 - shift],
                    cm_t[:, m:m + 1],
                    src[:, 1024:2048],
                    AluOpType.mult,
                    AluOpType.add,
                )
                src, dst = dst, src
            P = src  # (NH, 2048); S_j = P_{j-1} -> read at offset 960
            Pr = auxp.tile([NH, 1024], F32R, tag="Pr")
            nc.scalar.copy(Pr[:], P[:, 960:1984])

            # ---- per head: correction + within-chunk decay + normalize ----
            for h in range(NH):
                pk = psp.tile([128, NJ, HS], F32, tag="ps")
                for half in range(2):
                    nc.tensor.matmul(
                        pk[:, half * 8:(half + 1) * 8, :],
                        _r(dv_t[:, h * L:(h + 1) * L]),
                        _r(Pr[:, half * 512:(half + 1) * 512]),
                        start=True,
                        stop=False,
                    )
                    nc.tensor.matmul(
                        pk[:, half * 8:(half + 1) * 8, :],
                        _r(ct_t[:, h * 128:(h + 1) * 128]),
                        _r(k_all[:, half * 8:(half + 1) * 8, h * 64:(h + 1) * 64]),
                        start=False,
                        stop=True,
                    )
                sq = normp.tile([128, NJ, HS], F32, tag="sq")
                nc.scalar.square(sq[:], pk[:])
                ss = normp.tile([128, NJ], F32, tag="ss")
                nc.vector.tensor_reduce(
                    ss[:], sq[:], mybir.AxisListType.X, AluOpType.add
                )
                # sqrt(ss / scale_h^2) = norm / scale_h ; then reciprocal
                rns = normp.tile([128, NJ], F32, tag="rns")
                nc.scalar.activation(rns[:], ss[:], ACT.Sqrt, scale=float(invsc2[h]))
                rcp = normp.tile([128, NJ], F32, tag="rcp")
                nc.vector.reciprocal(rcp[:], rns[:])
                oh = outsp.tile([128, NJ, HS], F32, tag="oh")
                for j in range(NJ):
                    nc.vector.tensor_scalar_mul(
                        oh[:, j, :], pk[:, j, :], rcp[:, j:j + 1]
                    )
                nc.sync.dma_start(
                    outd[h].rearrange("(j r) d -> r j d", r=128), oh[:]
                )

    nc.compile()
    return nc


def _make_runner(nc):
    """Build a cached jax.jit shard_map runner for the SPMD kernel so
    repeated calls skip re-tracing (run_bass_via_pjrt rebuilds its jit
    closure on every call)."""
    b2j.install_neuronx_cc_hook()
    partition_name = nc.partition_id_tensor.name if nc.partition_id_tensor else None
    in_names, out_names, out_avals, zero_shapes = [], [], [], []
    for alloc in nc.m.functions[0].allocations:
        if not isinstance(alloc, mybir.MemoryLocationSet):
            continue
        name = alloc.memorylocations[0].name
        if alloc.kind == "ExternalInput":
            if name != partition_name:
                in_names.append(name)
        elif alloc.kind == "ExternalOutput":
            out_names.append(name)
            shape = tuple(alloc.tensor_shape)
            dtype = mybir.dt.np(alloc.dtype)
            out_avals.append(jax.core.ShapedArray(shape, dtype))
            zero_shapes.append((shape, dtype))
    n_params = len(in_names)
    n_outs = len(out_avals)
    all_in_names = list(in_names) + list(out_names)
    if partition_name is not None:
        all_in_names.append(partition_name)
    donate = tuple(range(n_params, n_params + n_outs))

    def _body(*args):
        operands = list(args)
        if partition_name is not None:
            operands.append(b2j.partition_id_tensor())
        outs = b2j._bass_exec_p.bind(
            *operands,
            out_avals=tuple(out_avals),
            in_names=tuple(all_in_names),
            out_names=tuple(out_names),
            lowering_input_output_aliases=(),
            sim_require_finite=True,
            sim_require_nnan=True,
            nc=nc,
        )
        return tuple(outs)

    devices = jax.devices()[:B]
    mesh = Mesh(np.asarray(devices), ("core",))
    in_specs = (PartitionSpec("core"),) * (n_params + n_outs)
    out_specs = (PartitionSpec("core"),) * n_outs
    sharded = jax.jit(
        shard_map(_body, mesh=mesh, in_specs=in_specs,
                  out_specs=out_specs, check_rep=False),
        donate_argnums=donate,
        keep_unused=True,
    )
    return sharded, in_names, out_names, zero_shapes


def _host_tensors(x, Wk_weight, betas):
    s = np.arange(L)
    diff = s[None, :] - s[:, None]  # [s, t] -> t - s
    ct = np.zeros((L, NH * L), np.float32)
    wv = np.zeros((L, NH), np.float32)
    dv = np.zeros((NH, NH * L), np.float32)  # masked: row h' nonzero iff h'==h
    cm = np.zeros((NH, 4), np.float32)
    for h in range(NH):
        b = betas[h]
        ct[:, h * L:(h + 1) * L] = np.where(diff >= 0, np.exp(-b * diff), 0.0)
        wv[:, h] = np.exp(-b * (L - 1 - s))
        dv[h, h * L:(h + 1) * L] = np.exp(-b * (s + 1.0))
        for m in range(4):
            cm[h, m] = np.exp(-b * L * (2.0 ** m))
    wkT = np.ascontiguousarray(Wk_weight.T).astype(np.float32)
    return ct, wv, dv, cm, wkT


def kernel(x, Wk_weight, leaky_key_beta, key_scale, scale_pow, **_):
    global LAST_RESULTS
    x = np.asarray(x, dtype=np.float32)
    Wk_weight = np.asarray(Wk_weight, dtype=np.float32)
    betas = (np.abs(np.asarray(leaky_key_beta, dtype=np.float64)).reshape(NH)
             * EXP_SCALING)
    ksc = np.asarray(key_scale, dtype=np.float64).reshape(NH)
    sp = float(np.asarray(scale_pow))
    scale = np.exp(np.minimum(sp * EXP_SCALING * ksc, KEY_SCALE_MAX))
    invsc2 = 1.0 / (scale * scale)

    key = (betas.tobytes(), invsc2.tobytes())
    if key not in _cache:
        nc = _build(invsc2)
        _cache[key] = (nc, _make_runner(nc))
    nc, (sharded, in_names, out_names, zero_shapes) = _cache[key]

    ct, wv, dv, cm, wkT = _host_tensors(x, Wk_weight, betas)
    per_core = {
        "xT": np.ascontiguousarray(np.transpose(x, (0, 2, 1))).reshape(B * C, T),
        "wkT": np.tile(wkT, (B, 1)),
        "ct": np.tile(ct, (B, 1)),
        "wv": np.tile(wv, (B, 1)),
        "dv": np.tile(dv, (B, 1)),
        "cm": np.tile(cm, (B, 1)),
    }
    concat_in = [per_core[name] for name in in_names]
    concat_zeros = [
        np.zeros((B * shape[0], *shape[1:]), dtype) for shape, dtype in zero_shapes
    ]
    out_arrs = sharded(*concat_in, *concat_zeros)
    out = np.asarray(out_arrs[out_names.index("out")])
    return out.reshape(B, NH, T, HS)
